# revision 6
# baseline (speedup 1.0000x reference)
"""GCN 2-layer encoder on 8 Trainium2 NeuronCores (Bass/Tile).

Sharding: nodes in 8 contiguous blocks of 12500 (dst-owner aggregates).
Per layer per core: h = x_local @ W (PE, bf16), y = dinv*h published as a
bf16 table via per-quarter AllGather; edge messages fetched with
dma_gather (int16 idx => 4 node-quarters per core-slice, tables <= 25600
rows, 256B bf16 rows); aggregation = one-hot (tensor_scalar is_equal,
bf16) matmuls accumulating in PSUM dst-windows of 128 nodes,
window-blocks of 12 windows double-buffered across 6 PSUM banks;
self-loop added at window flush (f32 accumulate path end to end:
PSUM f32 -> B f32 -> bias/scale f32; messages and tables bf16).

All CPU-side work is integer graph partitioning/relabeling (sharding);
every float op (including f32->bf16 casts) runs on device.
"""
import os
import sys

sys.path.insert(0, "/opt/trn_rl_repo")
import numpy as np

import concourse.tile as tile
from concourse import bacc, mybir, library_config
from concourse.bass_utils import run_bass_kernel_spmd

N_NODES = 100000
N_CORES = 8
S = N_NODES // N_CORES          # 12500 nodes per core
D = 128
NW = (S + 127) // 128           # 98 dst windows per core
WPT = 12                        # windows per window-block
NWB = (NW + WPT - 1) // WPT     # 9 window-blocks
QB = [0, 3200, 6400, 9472, S]   # quarter boundaries (tile-aligned starts)
QLEN = [QB[i + 1] - QB[i] for i in range(4)]
CALL = 1024                     # rows per dma_gather (HW-safe max)
f32 = mybir.dt.float32
bf16 = mybir.dt.bfloat16
i16 = mybir.dt.int16


def _quarter_of(m):
    q = np.searchsorted(np.array(QB[1:]), m, side="right")
    return q


def _prep(edge_index):
    """Integer-only graph partitioning -> common SPMD schedule + per-core
    idx/dst arrays. Returns (sched, per_core, deg)."""
    src = np.asarray(edge_index[0], dtype=np.int64)
    dst = np.asarray(edge_index[1], dtype=np.int64)
    deg = np.bincount(dst, minlength=N_NODES).astype(np.int64) + 1

    core = dst // S
    md = dst % S
    w = md // 128                       # window within core
    wb = w // WPT                       # window block
    ms = src % S
    q = _quarter_of(ms)                 # src quarter
    cs = src // S
    idx16 = cs * np.array(QLEN)[q] + (ms - np.array(QB)[q])  # row in table_q

    # per (core, wb, q) segment, edges sorted by (dst, src)
    segs_edges = {}
    order = np.lexsort((src, dst, q, wb, core))
    coreo, wbo, qo = core[order], wb[order], q[order]
    mdo, idxo = md[order], idx16[order]
    key = ((coreo * NWB + wbo) * 4 + qo)
    bounds = np.flatnonzero(np.diff(key)) + 1
    starts = np.concatenate([[0], bounds])
    ends = np.concatenate([bounds, [len(key)]])
    for s0, e0 in zip(starts, ends):
        segs_edges[int(key[s0])] = (mdo[s0:e0], idxo[s0:e0])

    # common segment lengths
    seg_list = [(b, qq) for b in range(NWB) for qq in range(4)]
    L = {}
    for (b, qq) in seg_list:
        mx = 0
        for c in range(N_CORES):
            k = (c * NWB + b) * 4 + qq
            if k in segs_edges:
                mx = max(mx, len(segs_edges[k][0]))
        L[(b, qq)] = max(128, ((mx + 127) // 128) * 128)

    tot_slots = sum(L.values())
    n_chunk_tot = tot_slots // 128

    # per-core padded arrays: gather idx (slot-major) and dstm per slot
    gidx_flat = np.zeros((N_CORES, tot_slots), dtype=np.int16)
    dstm_flat = np.full((N_CORES, tot_slots), -100000.0, dtype=np.float32)
    seg_base = {}
    off = 0
    for (b, qq) in seg_list:
        seg_base[(b, qq)] = off
        for c in range(N_CORES):
            k = (c * NWB + b) * 4 + qq
            if k in segs_edges:
                mdl, idxl = segs_edges[k]
                n = len(mdl)
                gidx_flat[c, off:off + n] = idxl.astype(np.int16)
                dstm_flat[c, off:off + n] = mdl.astype(np.float32)
        off += L[(b, qq)]

    # chunk -> union of touched windows across cores; op list
    ops = []
    first_op = {}
    last_op = {}
    for (b, qq) in seg_list:
        base = seg_base[(b, qq)]
        nch = L[(b, qq)] // 128
        for j in range(nch):
            sl = slice(base + 128 * j, base + 128 * (j + 1))
            vals = dstm_flat[:, sl]
            real = vals >= 0
            if not real.any():
                continue
            wins = np.unique((vals[real] // 128).astype(np.int64))
            for wv in wins:
                wv = int(wv)
                oi = len(ops)
                ops.append([b, qq, j, wv, False, False])
                if (b, wv) not in first_op:
                    first_op[(b, wv)] = oi
                last_op[(b, wv)] = oi
    # start/stop once per PSUM *bank* per window-block
    first_bk, last_bk = {}, {}
    for oi, (b, qq, j, wv, _, _) in enumerate(ops):
        bk = (b, (wv - b * WPT) // 4)
        if bk not in first_bk:
            first_bk[bk] = oi
        last_bk[bk] = oi
    for oi in first_bk.values():
        ops[oi][4] = True
    for oi in last_bk.values():
        ops[oi][5] = True

    # dstm per chunk column (device compare: (iota - dstm) == -128*w)
    dstmT = np.empty((N_CORES, 128, n_chunk_tot), dtype=np.float32)
    for jg in range(n_chunk_tot):
        dstmT[:, :, jg] = dstm_flat[:, 128 * jg:128 * (jg + 1)]

    # wrap gather idx: [128, tot/16], idx i at (i%16, i//16), 8x replicated
    gidx_w = np.empty((N_CORES, 128, tot_slots // 16), dtype=np.int16)
    for c in range(N_CORES):
        a = gidx_flat[c].reshape(-1, 16).T        # [16, tot/16]
        gidx_w[c] = np.tile(a, (8, 1))

    sched = {
        "L": L, "seg_list": seg_list, "seg_base": seg_base,
        "ops": ops, "tot_slots": tot_slots, "n_chunk_tot": n_chunk_tot,
        "windows_per_wb": [min(NW - b * WPT, WPT) for b in range(NWB)],
        "first_op": first_op, "last_op": last_op,
    }
    return sched, gidx_w, dstmT, deg


def _build(sched, repeat=1, call=CALL, php_bufs=2, oh_bufs=8, stg_bufs=6):
    """bf16 message path: tables/gather/one-hot matmuls in bf16; repeat
    is python-unrolled so the AllGathers stay inside every iteration."""
    nc = bacc.Bacc("TRN2", target_bir_lowering=False, debug=False,
                   num_devices=N_CORES)
    NCOL = NW * 128
    xT = nc.dram_tensor("xT", [128, NCOL], f32, kind="ExternalInput")
    W1 = nc.dram_tensor("W1", [128, 128], f32, kind="ExternalInput")
    W2 = nc.dram_tensor("W2", [128, 128], f32, kind="ExternalInput")
    b1b = nc.dram_tensor("b1b", [128, 128], f32, kind="ExternalInput")
    b2b = nc.dram_tensor("b2b", [128, 128], f32, kind="ExternalInput")
    degi = nc.dram_tensor("degi", [128, NW], f32, kind="ExternalInput")
    iotad = nc.dram_tensor("iotad", [128, 128], f32, kind="ExternalInput")
    identd = nc.dram_tensor("identd", [128, 128], f32, kind="ExternalInput")
    gidx = nc.dram_tensor("gidx", [128, sched["tot_slots"] // 16], i16,
                          kind="ExternalInput")
    dstmT = nc.dram_tensor("dstmT", [128, sched["n_chunk_tot"]], f32,
                           kind="ExternalInput")
    out = nc.dram_tensor("out", [S, D], f32, kind="ExternalOutput")

    y_slice = nc.dram_tensor("y_slice", [S, D], bf16)
    tables = [nc.dram_tensor(f"table{qq}", [N_CORES * QLEN[qq], D], bf16,
                             addr_space="Shared") for qq in range(4)]

    seg_list, L, seg_base = sched["seg_list"], sched["L"], sched["seg_base"]
    ops, wpwb = sched["ops"], sched["windows_per_wb"]
    ops_by_seg = {sk: [] for sk in seg_list}
    for op in ops:
        ops_by_seg[(op[0], op[1])].append(op)

    with tile.TileContext(nc) as tc:
        with (
            tc.tile_pool(name="cst", bufs=1) as cst,
            tc.tile_pool(name="big", bufs=1) as big,
            tc.tile_pool(name="st", bufs=stg_bufs) as stp,
            tc.tile_pool(name="oh", bufs=oh_bufs) as ohp,
            tc.tile_pool(name="bank", bufs=1, space="PSUM") as bankp,
            tc.tile_pool(name="php", bufs=php_bufs, space="PSUM") as php,
            tc.tile_pool(name="tmp", bufs=3) as tmp,
        ):
            nc.gpsimd.load_library(library_config.mlp)

            xT16 = cst.tile([128, NCOL], bf16, tag="xT16")
            W1_sb = cst.tile([128, 128], bf16, tag="W1")
            W2_sb = cst.tile([128, 128], bf16, tag="W2")
            b1_sb = cst.tile([128, 128], f32, tag="b1")
            b2_sb = cst.tile([128, 128], f32, tag="b2")
            deg_sb = cst.tile([128, NW], f32, tag="deg")
            dinv_sb = cst.tile([128, NW], f32, tag="dinv")
            iota_sb = cst.tile([128, 128], bf16, tag="iota")
            id_sb = cst.tile([128, 128], f32, tag="ident")
            gidx_sb = cst.tile([128, sched["tot_slots"] // 16], i16, tag="gx")
            dstm_sb = cst.tile([128, sched["n_chunk_tot"]], f32, tag="dm")
            A = big.tile([128, NCOL], bf16, tag="A")   # y (bf16 messages)
            B = big.tile([128, NCOL], f32, tag="B")    # aggregation acc
            banks = [bankp.tile([128, 512], f32, tag=f"bk{i}",
                                name=f"bank{i}")
                     for i in range(6)]

            # f32 inputs -> bf16 SBUF tiles (device-side cast)
            for t in range(NW):
                csl = slice(128 * t, 128 * (t + 1))
                xt_f = tmp.tile([128, 128], f32, tag="xtf")
                nc.sync.dma_start(xt_f[:], xT[:, csl])
                nc.vector.tensor_copy(xT16[:, csl], xt_f[:])
            w_f = tmp.tile([128, 128], f32, tag="xtf")
            nc.sync.dma_start(w_f[:], W1[:])
            nc.vector.tensor_copy(W1_sb[:], w_f[:])
            w2f = tmp.tile([128, 128], f32, tag="xtf")
            nc.sync.dma_start(w2f[:], W2[:])
            nc.vector.tensor_copy(W2_sb[:], w2f[:])
            nc.sync.dma_start(id_sb[:], identd[:])
            nc.sync.dma_start(b1_sb[:], b1b[:])
            nc.sync.dma_start(b2_sb[:], b2b[:])
            nc.sync.dma_start(deg_sb[:], degi[:])
            iof = tmp.tile([128, 128], f32, tag="xtf")
            nc.sync.dma_start(iof[:], iotad[:])
            nc.vector.tensor_copy(iota_sb[:], iof[:])
            nc.sync.dma_start(gidx_sb[:], gidx[:])
            nc.sync.dma_start(dstm_sb[:], dstmT[:])
            nc.vector.reciprocal(dinv_sb[:], deg_sb[:])
            nc.scalar.activation(dinv_sb[:], dinv_sb[:],
                                 mybir.ActivationFunctionType.Sqrt)

            def publish_quarters():
                for qq in range(4):
                    r0, r1 = QB[qq], QB[qq + 1]
                    t0, p0 = r0 // 128, r0 % 128
                    t1, p1 = r1 // 128, r1 % 128
                    assert p0 == 0
                    if t1 > t0:
                        nc.sync.dma_start(
                            y_slice.ap()[r0:128 * t1, :].rearrange(
                                "(t p) f -> p t f", p=128),
                            A[:, 128 * t0:128 * t1].rearrange(
                                "p (t f) -> p t f", f=128))
                    if p1:
                        nc.sync.dma_start(
                            y_slice.ap()[128 * t1:r1, :],
                            A[0:p1, 128 * t1:128 * (t1 + 1)])
                for qq in range(4):
                    nc.gpsimd.collective_compute(
                        "AllGather", mybir.AluOpType.bypass,
                        replica_groups=[list(range(N_CORES))],
                        ins=[y_slice.ap()[QB[qq]:QB[qq + 1], :].opt()],
                        outs=[tables[qq].ap().opt()])

            def aggregate():
                for b in range(NWB):
                    nwin = wpwb[b]
                    for qq in range(4):
                        base = seg_base[(b, qq)]
                        Lseg = L[(b, qq)]
                        ncalls = (Lseg + call - 1) // call
                        stages = []
                        for k in range(ncalls):
                            cl = min(call, Lseg - call * k)
                            stg = stp.tile([128, call // 128, 128], bf16,
                                           tag="stg")
                            nc.gpsimd.dma_gather(
                                stg[:, :cl // 128, :], tables[qq].ap(),
                                gidx_sb[:, (base + call * k) // 16:
                                        (base + call * k + cl) // 16],
                                cl, cl, 128)
                            stages.append(stg)
                        for op in ops_by_seg[(b, qq)]:
                            _, _, j, wv, st_f, sp_f = op
                            jg = (base // 128) + j
                            k, jc = j // (call // 128), j % (call // 128)
                            S_t = ohp.tile([128, 128], bf16, tag="S")
                            nc.vector.tensor_scalar(
                                S_t[:], iota_sb[:],
                                dstm_sb[:, jg:jg + 1], float(-128.0 * wv),
                                op0=mybir.AluOpType.subtract,
                                op1=mybir.AluOpType.is_equal)
                            wl = wv - b * WPT
                            bank = banks[(b % 2) * 3 + wl // 4]
                            bsl = bank[:, 128 * (wl % 4):128 * (wl % 4 + 1)]
                            nc.tensor.matmul(
                                bsl, lhsT=S_t[:], rhs=stages[k][:, jc, :],
                                start=st_f, stop=sp_f)
                    # flush this window block
                    for wl in range(nwin):
                        wv = b * WPT + wl
                        csl = slice(128 * wv, 128 * (wv + 1))
                        bank = banks[(b % 2) * 3 + wl // 4]
                        bsl = bank[:, 128 * (wl % 4):128 * (wl % 4 + 1)]
                        if (b, wv) in sched["first_op"]:
                            nc.vector.tensor_tensor(
                                B[:, csl], bsl, A[:, csl],
                                op=mybir.AluOpType.add)
                        else:
                            nc.vector.tensor_copy(B[:, csl], A[:, csl])

            for _rep in range(repeat):
                # ---------------- layer 1 ----------------
                for t in range(NW):
                    h_ps = php.tile([128, 128], f32, tag="php")
                    nc.tensor.matmul(h_ps[:],
                                     lhsT=xT16[:, 128 * t:128 * (t + 1)],
                                     rhs=W1_sb[:], start=True, stop=True)
                    nc.vector.tensor_scalar(A[:, 128 * t:128 * (t + 1)],
                                            h_ps[:],
                                            dinv_sb[:, t:t + 1], None,
                                            op0=mybir.AluOpType.mult)
                publish_quarters()
                aggregate()
                # out1 = B*dinv + b1  (z = relu(out1) formed per-tile in
                # layer 2: transpose first, relu+bf16-cast on Act engine)
                for t in range(NW):
                    csl = slice(128 * t, 128 * (t + 1))
                    nc.vector.tensor_scalar(B[:, csl], B[:, csl],
                                            dinv_sb[:, t:t + 1], None,
                                            op0=mybir.AluOpType.mult)
                    nc.vector.tensor_tensor(B[:, csl], B[:, csl], b1_sb[:],
                                            op=mybir.AluOpType.add)

                # ---------------- layer 2 ----------------
                for t in range(NW):
                    csl = slice(128 * t, 128 * (t + 1))
                    t_ps = php.tile([128, 128], f32, tag="php")
                    nc.tensor.transpose(t_ps[:], B[:, csl], id_sb[:])
                    zT = tmp.tile([128, 128], bf16, tag="zT")
                    nc.scalar.activation(zT[:], t_ps[:],
                                         mybir.ActivationFunctionType.Relu)
                    h_ps = php.tile([128, 128], f32, tag="php")
                    nc.tensor.matmul(h_ps[:], lhsT=zT[:], rhs=W2_sb[:],
                                     start=True, stop=True)
                    nc.vector.tensor_scalar(A[:, csl], h_ps[:],
                                            dinv_sb[:, t:t + 1], None,
                                            op0=mybir.AluOpType.mult)
                publish_quarters()
                aggregate()
                for t in range(NW):
                    csl = slice(128 * t, 128 * (t + 1))
                    nc.vector.tensor_scalar(B[:, csl], B[:, csl],
                                            dinv_sb[:, t:t + 1], None,
                                            op0=mybir.AluOpType.mult)
                    nc.vector.tensor_tensor(B[:, csl], B[:, csl], b2_sb[:],
                                            op=mybir.AluOpType.add)
                # write output rows 0..12500
                nc.sync.dma_start(
                    out.ap()[0:128 * 97, :].rearrange("(t p) f -> p t f",
                                                      p=128),
                    B[:, 0:128 * 97].rearrange("p (t f) -> p t f", f=128))
                nc.sync.dma_start(out.ap()[128 * 97:S, :],
                                  B[0:S - 128 * 97, 128 * 97:128 * 98])

    nc.compile()
    return nc


def _make_in_maps(x, W1, b1, W2, b2, sched, gidx_w, dstmT, deg):
    NCOL = NW * 128
    iota = np.broadcast_to(np.arange(128, dtype=np.float32),
                           (128, 128)).copy()
    ident = np.eye(128, dtype=np.float32)
    in_maps = []
    for c in range(N_CORES):
        xs = x[S * c:S * (c + 1)].astype(np.float32)
        xT = np.zeros((128, NCOL), np.float32)
        xT[:, :S] = xs.T
        degc = deg[S * c:S * (c + 1)].astype(np.float32)
        degp = np.ones(NCOL, np.float32)
        degp[:S] = degc
        # node n -> [n%128, n//128]
        deg_pc = degp.reshape(NW, 128).T.copy()
        in_maps.append({
            "xT": xT,
            "W1": W1.astype(np.float32), "W2": W2.astype(np.float32),
            "b1b": np.broadcast_to(b1.astype(np.float32), (128, 128)).copy(),
            "b2b": np.broadcast_to(b2.astype(np.float32), (128, 128)).copy(),
            "degi": deg_pc, "iotad": iota, "identd": ident,
            "gidx": gidx_w[c], "dstmT": dstmT[c],
        })
    return in_maps


def kernel(x, edge_index, W1, b1, W2, b2):
    sched, gidx_w, dstmT, deg = _prep(edge_index)
    nc = _build(sched, repeat=int(os.environ.get("KERNEL_REPEAT", "1")))
    in_maps = _make_in_maps(x, W1, b1, W2, b2, sched, gidx_w, dstmT, deg)
    res = run_bass_kernel_spmd(nc, in_maps, core_ids=list(range(N_CORES)))
    return np.concatenate([res.results[c]["out"] for c in range(N_CORES)], 0)
